# revision 8
# baseline (speedup 1.0000x reference)
# Trainium2 Bass kernel for nn_MinLoss_15229954032079.
#
# Math: loss = sum_b sum_s dist(p[b,s], g[b,match(b,s)]) / B, where
# dist is the euclidean distance between flattened [T*D] source signals
# and match is a greedy bipartite assignment on the [S,S] distance matrix.
#
# All pairwise distances derive from the 8x8 Gram matrix of the 8 flattened
# source vectors (4 prediction sources + 4 ground-truth sources) per batch:
#   d2[s,t] = G[s,s] + G[4+t,4+t] - 2*G[s,4+t]
#
# Strategy (one NeuronCore per batch element, 8 cores):
#   - The per-core stream (33.7 MB f32) is bound by per-SDMA-engine
#     bandwidth (~26 GB/s read x 16 engines). SWDGE f32->bf16 cast
#     landings keep the SBUF write side at half the read side.
#   - SDMA engine 15 runs ~10% slower than engines 0-14 on the SWDGE
#     path (descriptor-ring port contention). It serves exactly SBUF
#     partitions {92-95, 124-127}. The row->partition assignment is a
#     free choice (the Gram sums over everything), so those 8 partitions
#     get 28 time rows instead of 32: they only participate in the seven
#     uniform 512-row windows. The displaced 512 rows stream in extra
#     windows that land on partitions [0:92)+[96:124) only (engines
#     0-14), sized so every engine finishes within ~1 us.
#   - Per window, DVE shuffles the p-half and ACT (scalar) the g-half
#     into a blocked bf16 layout: block r=(ti,dg) holds one column group
#     of 16 consecutive d's per source j, so every matmul operand is a
#     contiguous 128-column slice. Partition lanes never written in the
#     ragged windows are pre-zeroed (zeros add nothing to the Gram).
#   - For each 128-column block, accumulate PSUM += block^T @ block.
#     PSUM entry (16j+u, 16j'+u) holds partial dot products of sources
#     j,j'; summing the 16 u-diagonals on the host yields the exact 8x8
#     Gram. The d=256 leftover columns accumulate into a second [32,32]
#     PSUM at col = t*8 + half*4 + j.
#   - The ragged windows taper (240, 120, 120 rows) so the serial tail
#     after the last DMA byte is one small window's copies + 16 matmuls.
#   - Tiny [4,4] greedy matching + final scalar reduction on host.
#   - TileContext's exit sequence is patched to skip the per-semaphore
#     clear pass (each run executes a freshly loaded NEFF).

import numpy as np

B, T, S, D = 8, 4096, 4, 257
NCORES = 8
PSB = 32  # tail psum operand width: col = t*8 + h*4 + j (t<4)

# Window plan: list of (segments, dedicated_wb). Each segment is
# (partition_start, partition_count, ti); a segment reads
# partition_count*ti consecutive DRAM rows per tensor. ti is uniform
# within a window. Issue order matters: w512#0 first so engines 1-13
# (odd) start streaming immediately; wX (extra rows for partitions
# [0:32), engines 0-14 even) second so it hides in the stream.
W512 = [(0, 128, 4)]
WX = [(0, 32, 1)]
WF2 = [(0, 92, 2), (96, 28, 2)]
WF1 = [(0, 92, 1), (96, 28, 1)]
WINDOWS = [W512, WX] + [W512] * 6 + [WF2, WF1, WF1]
NMM = 16 * sum(w[0][2] for w in WINDOWS)

_cached_nc = None


def _light_drain_and_barrier(self, tick_clock, wait_clock):
    # Replaces TileContext._drain_and_barrier: keep the drain + one
    # all-engine barrier, but skip the per-semaphore clear pass and the
    # second barrier (~6 us). Safe here because every kernel() invocation
    # executes a freshly loaded NEFF, so semaphores start from zero and
    # don't need to be restored for a re-run.
    from concourse.vector_clock import ScopedClock

    drain_inst = self.nc.sync.drain()
    wait_clock.add_sem_waits(
        drain_inst.ins, ScopedClock({None: tick_clock.global_clock})
    )
    self.nc.all_engine_barrier()
    popped = self.nc._tile_sem_poison_stack.pop()
    assert popped is self._sem_poison


def _build_nc():
    import concourse.bacc as bacc
    import concourse.tile as tile
    from concourse import mybir

    nc = bacc.Bacc("TRN2", target_bir_lowering=False, debug=False)
    p_dram = nc.dram_tensor("p", [T, S, D], mybir.dt.float32, kind="ExternalInput")
    g_dram = nc.dram_tensor("g", [T, S, D], mybir.dt.float32, kind="ExternalInput")
    gram_dram = nc.dram_tensor(
        "gram", [128, 128 + PSB], mybir.dt.float32, kind="ExternalOutput"
    )

    orig_drain = tile.TileContext._drain_and_barrier
    tile.TileContext._drain_and_barrier = _light_drain_and_barrier

    with tile.TileContext(nc) as tc:
        n_ragged = sum(1 for w in WINDOWS if w is not W512)
        with (
            tc.tile_pool(name="slab", bufs=6) as fpool,
            tc.tile_pool(name="blk16", bufs=3) as bpool,
            tc.tile_pool(name="ded", bufs=1) as dpool,
            tc.tile_pool(name="psum", bufs=1, space="PSUM") as ppool,
            tc.tile_pool(name="out", bufs=1) as opool,
        ):
            psa = ppool.tile([128, 128], mybir.dt.float32)
            psb = ppool.tile([PSB, PSB], mybir.dt.float32)

            # Dedicated pre-zeroed wb tiles for ragged windows: partition
            # lanes their copies never write stay zero and contribute
            # nothing to the PSUM accumulation.
            ded_wb = {}
            for wi, segs in enumerate(WINDOWS):
                if segs is W512:
                    continue
                ti = segs[0][2]
                wb = dpool.tile(
                    [128, 128 * 16 * ti + PSB], mybir.dt.bfloat16, name=f"wbd{wi}"
                )
                nc.vector.memset(wb[:], 0.0)
                ded_wb[wi] = wb

            mm_i = 0
            row = 0
            for wi, segs in enumerate(WINDOWS):
                ti = segs[0][2]
                half = ti * S * D
                nblk = 16 * ti
                # fixed-size slab (pool slots rotate across same-named tiles)
                fslf = fpool.tile([128, 2 * 4 * S * D], mybir.dt.bfloat16, name="fsl")
                fsl = fslf[:, 0 : 2 * half]
                seg_rows = []
                for ps, pcnt, _ in segs:
                    seg_rows.append(row)
                    row += pcnt * ti
                for tensor_i, dram in enumerate((p_dram, g_dram)):
                    for (ps, pcnt, _), r0 in zip(segs, seg_rows):
                        nrows = pcnt * ti
                        src = dram.ap()[r0 : r0 + nrows].rearrange(
                            "(p ti) s d -> p ti s d", p=pcnt
                        )
                        dst = fsl[ps : ps + pcnt, tensor_i * half : tensor_i * half + half]
                        nc.gpsimd.dma_start(out=dst, in_=src)
                wb = ded_wb.get(wi)
                if wb is None:
                    wbf = bpool.tile(
                        [128, 128 * 64 + PSB], mybir.dt.bfloat16, name="wb"
                    )
                    wb = wbf[:, 0 : 128 * nblk + PSB]
                wv = wb[:, 0 : 128 * nblk].rearrange(
                    "p (ti dg j dl) -> p j ti dg dl", ti=ti, dg=16, j=8, dl=16
                )
                tb = wb[:, 128 * nblk : 128 * nblk + PSB].rearrange(
                    "p (t h j) -> p t h j", t=4, h=2, j=4
                )
                for h in (0, 1):
                    for ps, pcnt, _ in segs:
                        hview = fsl[ps : ps + pcnt, h * half : (h + 1) * half].rearrange(
                            "p (ti c) -> p ti c", ti=ti
                        )
                        for jj in range(4):
                            src = hview[:, :, jj * D : jj * D + 256].rearrange(
                                "p ti (dg dl) -> p ti dg dl", dl=16
                            )
                            dst = wv[ps : ps + pcnt, h * 4 + jj]
                            if h == 0:
                                nc.vector.tensor_copy(dst, src)
                            else:
                                nc.scalar.copy(dst, src)
                        tsrc = hview.rearrange("p ti (j d) -> p ti j d", j=4)[
                            :, :, :, 256
                        ]
                        tdst = tb[ps : ps + pcnt, 0:ti, h]
                        if h == 0:
                            nc.vector.tensor_copy(tdst, tsrc)
                        else:
                            nc.scalar.copy(tdst, tsrc)
                for r in range(nblk):
                    blk = wb[:, 128 * r : 128 * (r + 1)]
                    nc.tensor.matmul(
                        psa[:], blk, blk, start=(mm_i == 0), stop=(mm_i == NMM - 1)
                    )
                    mm_i += 1
                tblk = wb[:, 128 * nblk : 128 * nblk + PSB]
                nc.tensor.matmul(
                    psb[:], tblk, tblk, start=(wi == 0), stop=(wi == len(WINDOWS) - 1)
                )

            outt = opool.tile([128, 128 + PSB], mybir.dt.float32)
            nc.vector.tensor_copy(outt[:, 0:128], psa[:])
            nc.vector.tensor_copy(outt[0:PSB, 128 : 128 + PSB], psb[:])
            nc.sync.dma_start(out=gram_dram.ap(), in_=outt[:])
    tile.TileContext._drain_and_barrier = orig_drain
    nc.compile()
    return nc


def _greedy_match_np(d):
    # replicate reference._greedy_match: repeated global argmin with
    # row/col masking; np.argmin matches jnp.argmin tie-breaking (first).
    s = d.shape[0]
    dm = d.astype(np.float32).copy()
    matches = np.zeros(s, np.int32)
    for _ in range(s):
        m = int(np.argmin(dm.reshape(-1)))
        r, c = divmod(m, s)
        matches[r] = c
        dm[r, :] = np.inf
        dm[:, c] = np.inf
    return matches


def _loss_from_gram(res_list):
    total = 0.0
    for out in res_list:
        psa = out[:, 0:128]
        psb = out[0:PSB, 128 : 128 + PSB]
        # G8[j,k] = sum_u psa[16j+u, 16k+u] + sum_t psb[t*8+j, t*8+k]
        g8 = np.einsum("juku->jk", psa.reshape(8, 16, 8, 16).astype(np.float64))
        g8 += np.einsum("tjtk->jk", psb.reshape(4, 8, 4, 8).astype(np.float64))
        pn = np.diag(g8)[:4]
        gn = np.diag(g8)[4:]
        cr = g8[:4, 4:]
        d2 = pn[:, None] + gn[None, :] - 2.0 * cr
        dists = np.sqrt(np.maximum(d2, 0.0)).astype(np.float32)
        matches = _greedy_match_np(dists)
        total += float(dists[np.arange(4), matches].astype(np.float64).sum())
    return np.float32(total / B)


def kernel(**inputs):
    global _cached_nc
    preds = np.ascontiguousarray(inputs["predictions"], dtype=np.float32)
    gts = np.ascontiguousarray(inputs["ground_truths"], dtype=np.float32)
    assert preds.shape == (B, T, S, D) and gts.shape == (B, T, S, D)

    if _cached_nc is None:
        _cached_nc = _build_nc()
    nc = _cached_nc

    from concourse.bass_utils import run_bass_kernel_spmd

    in_maps = [{"p": preds[b], "g": gts[b]} for b in range(B)]
    res = run_bass_kernel_spmd(nc, in_maps, list(range(NCORES)))
    return _loss_from_gram([res.results[b]["gram"] for b in range(B)])


# revision 9
# speedup vs baseline: 1.1881x; 1.1881x over previous
# Trainium2 Bass kernel for nn_MinLoss_15229954032079.
#
# Math: loss = sum_b sum_s dist(p[b,s], g[b,match(b,s)]) / B, where
# dist is the euclidean distance between flattened [T*D] source signals
# and match is a greedy bipartite assignment on the [S,S] distance matrix.
#
# All pairwise distances derive from the 8x8 Gram matrix of the 8 flattened
# source vectors (4 prediction sources + 4 ground-truth sources) per batch:
#   d2[s,t] = G[s,s] + G[4+t,4+t] - 2*G[s,4+t]
#
# Strategy (one NeuronCore per batch element, 8 cores):
#   - The per-core stream (33.7 MB f32) is bound by per-SDMA-engine READ
#     bandwidth: ~26.4 GB/s x 16 engines => ~80 us floor. Measured: the
#     landing dtype (f32/bf16/fp8) does not change DMA time at all, and
#     HWDGE runs slower (~21.5 GB/s/engine), so the stream uses SWDGE
#     (gpsimd) f32->bf16 cast landings: same speed, half the SBUF.
#   - Windows taper (512x7, 256, 128, 128) so the serial tail after the
#     last DMA byte is one 128-row window's copies + 16 matmuls.
#   - Per window, the shuffle into matmul layout is split: DVE copies
#     sources j=0..4, ACT (scalar engine) j=5..7. Blocked bf16 layout:
#     block r=(ti,dg) holds one column group of 16 consecutive d's per
#     source j, so every matmul operand is a contiguous 128-column slice.
#   - For each 128-column block, accumulate PSUM += block^T @ block on
#     the PE. PSUM entry (16j+u, 16j'+u) holds partial dot products of
#     sources j,j'; summing the 16 u-diagonals on the host yields the
#     exact 8x8 Gram. The d=256 leftover columns go to a second [32,32]
#     PSUM at col = t*8 + half*4 + j (t<ti; unused cols of short windows
#     are zeroed so the fixed-width accumulation stays clean).
#   - Both PSUMs drain into one [128, 160] tile -> single output DMA.
#   - Tiny [4,4] greedy matching + final scalar reduction on host.
#   - TileContext's exit sequence is patched to skip the per-semaphore
#     clear pass (each run executes a freshly loaded NEFF).

import numpy as np

B, T, S, D = 8, 4096, 4, 257
NCORES = 8
WSIZES = [512] * 7 + [256, 128, 128]  # time steps per window; sum == T
NW = len(WSIZES)
NMM = 16 * (sum(WSIZES) // 128)  # total body matmuls
PSB = 32  # tail psum operand width: col = t*8 + h*4 + j (t<4)
DVE_J = 5  # sources 0..DVE_J-1 copied by DVE, rest by ACT

_cached_nc = None


def _light_drain_and_barrier(self, tick_clock, wait_clock):
    # Replaces TileContext._drain_and_barrier: keep the drain + one
    # all-engine barrier, but skip the per-semaphore clear pass and the
    # second barrier (~6 us). Safe here because every kernel() invocation
    # executes a freshly loaded NEFF, so semaphores start from zero and
    # don't need to be restored for a re-run.
    from concourse.vector_clock import ScopedClock

    drain_inst = self.nc.sync.drain()
    wait_clock.add_sem_waits(
        drain_inst.ins, ScopedClock({None: tick_clock.global_clock})
    )
    self.nc.all_engine_barrier()
    popped = self.nc._tile_sem_poison_stack.pop()
    assert popped is self._sem_poison


def _build_nc():
    import concourse.bacc as bacc
    import concourse.tile as tile
    from concourse import mybir

    nc = bacc.Bacc("TRN2", target_bir_lowering=False, debug=False, num_swdge_queues=2)
    p_dram = nc.dram_tensor("p", [T, S, D], mybir.dt.float32, kind="ExternalInput")
    g_dram = nc.dram_tensor("g", [T, S, D], mybir.dt.float32, kind="ExternalInput")
    gram_dram = nc.dram_tensor(
        "gram", [128, 128 + PSB], mybir.dt.float32, kind="ExternalOutput"
    )

    orig_drain = tile.TileContext._drain_and_barrier
    tile.TileContext._drain_and_barrier = _light_drain_and_barrier

    with tile.TileContext(nc) as tc:
        with (
            tc.tile_pool(name="slab", bufs=6) as fpool,
            tc.tile_pool(name="blk16", bufs=3) as bpool,
            tc.tile_pool(name="psum", bufs=1, space="PSUM") as ppool,
            tc.tile_pool(name="out", bufs=1) as opool,
        ):
            psa = ppool.tile([128, 128], mybir.dt.float32)
            psb = ppool.tile([PSB, PSB], mybir.dt.float32)

            mm_i = 0
            row = 0
            for w, ws in enumerate(WSIZES):
                ti = ws // 128
                half = ti * S * D
                nblk = 16 * ti
                fslf = fpool.tile([128, 2 * 4 * S * D], mybir.dt.bfloat16, name="fsl")
                fsl = fslf[:, 0 : 2 * half]
                for tensor_i, dram in enumerate((p_dram, g_dram)):
                    src = dram.ap()[row : row + ws].rearrange(
                        "(p ti) s d -> p ti s d", p=128
                    )
                    nc.gpsimd.dma_start(
                        out=fsl[:, tensor_i * half : (tensor_i + 1) * half], in_=src
                    )
                row += ws

                wbf = bpool.tile([128, 128 * 64 + PSB], mybir.dt.bfloat16, name="wb")
                wb = wbf[:, 0 : 128 * nblk + PSB]
                wv = wb[:, 0 : 128 * nblk].rearrange(
                    "p (ti dg j dl) -> p j ti dg dl", ti=ti, dg=16, j=8, dl=16
                )
                tb = wb[:, 128 * nblk : 128 * nblk + PSB].rearrange(
                    "p (t h j) -> p t h j", t=4, h=2, j=4
                )
                if ti < 4:
                    # zero unused tail cols of the fixed-width psb operand
                    nc.vector.memset(
                        wb[:, 128 * nblk + 8 * ti : 128 * nblk + PSB], 0.0
                    )
                for h in (0, 1):
                    hview = fsl[:, h * half : (h + 1) * half].rearrange(
                        "p (ti c) -> p ti c", ti=ti
                    )
                    for jj in range(4):
                        src = hview[:, :, jj * D : jj * D + 256].rearrange(
                            "p ti (dg dl) -> p ti dg dl", dl=16
                        )
                        dst = wv[:, h * 4 + jj]
                        if h * 4 + jj < DVE_J:
                            nc.vector.tensor_copy(dst, src)
                        else:
                            nc.scalar.copy(dst, src)
                    tsrc = hview.rearrange("p ti (j d) -> p ti j d", j=4)[:, :, :, 256]
                    tdst = tb[:, 0:ti, h]
                    if h == 0:
                        nc.vector.tensor_copy(tdst, tsrc)
                    else:
                        nc.scalar.copy(tdst, tsrc)
                for r in range(nblk):
                    blk = wb[:, 128 * r : 128 * (r + 1)]
                    nc.tensor.matmul(
                        psa[:], blk, blk, start=(mm_i == 0), stop=(mm_i == NMM - 1)
                    )
                    mm_i += 1
                tblk = wb[:, 128 * nblk : 128 * nblk + PSB]
                nc.tensor.matmul(
                    psb[:], tblk, tblk, start=(w == 0), stop=(w == NW - 1)
                )

            outt = opool.tile([128, 128 + PSB], mybir.dt.float32)
            nc.vector.tensor_copy(outt[:, 0:128], psa[:])
            nc.vector.tensor_copy(outt[0:PSB, 128 : 128 + PSB], psb[:])
            nc.sync.dma_start(out=gram_dram.ap(), in_=outt[:])
    tile.TileContext._drain_and_barrier = orig_drain
    nc.compile()
    return nc


def _greedy_match_np(d):
    # replicate reference._greedy_match: repeated global argmin with
    # row/col masking; np.argmin matches jnp.argmin tie-breaking (first).
    s = d.shape[0]
    dm = d.astype(np.float32).copy()
    matches = np.zeros(s, np.int32)
    for _ in range(s):
        m = int(np.argmin(dm.reshape(-1)))
        r, c = divmod(m, s)
        matches[r] = c
        dm[r, :] = np.inf
        dm[:, c] = np.inf
    return matches


def _loss_from_gram(res_list):
    total = 0.0
    for out in res_list:
        psa = out[:, 0:128]
        psb = out[0:PSB, 128 : 128 + PSB]
        # G8[j,k] = sum_u psa[16j+u, 16k+u] + sum_t psb[t*8+j, t*8+k]
        g8 = np.einsum("juku->jk", psa.reshape(8, 16, 8, 16).astype(np.float64))
        g8 += np.einsum("tjtk->jk", psb.reshape(4, 8, 4, 8).astype(np.float64))
        pn = np.diag(g8)[:4]
        gn = np.diag(g8)[4:]
        cr = g8[:4, 4:]
        d2 = pn[:, None] + gn[None, :] - 2.0 * cr
        dists = np.sqrt(np.maximum(d2, 0.0)).astype(np.float32)
        matches = _greedy_match_np(dists)
        total += float(dists[np.arange(4), matches].astype(np.float64).sum())
    return np.float32(total / B)


def kernel(**inputs):
    global _cached_nc
    preds = np.ascontiguousarray(inputs["predictions"], dtype=np.float32)
    gts = np.ascontiguousarray(inputs["ground_truths"], dtype=np.float32)
    assert preds.shape == (B, T, S, D) and gts.shape == (B, T, S, D)

    if _cached_nc is None:
        _cached_nc = _build_nc()
    nc = _cached_nc

    from concourse.bass_utils import run_bass_kernel_spmd

    in_maps = [{"p": preds[b], "g": gts[b]} for b in range(B)]
    res = run_bass_kernel_spmd(nc, in_maps, list(range(NCORES)))
    return _loss_from_gram([res.results[b]["gram"] for b in range(B)])
